# revision 2
# baseline (speedup 1.0000x reference)
"""Trainium2 Bass kernel for nn_Dist (retrieval_knn): for every pixel of a
B=2 x 64 x 192 grid, find the 4 nearest valid (sparse-depth) pixels, with
jax.lax.top_k tie-breaking (equal distance -> lower pixel index first).

Strategy
--------
Host marshaling: compact the ~5% valid candidate pixels per batch (order-
preserving, so candidate order == pixel-index order), and pick for every
128-query tile a 256-candidate window (contiguous slice of the y-sorted
candidate list) that maximizes the guard band of fully-covered rows around
the tile. Sharding: 8 cores = 2 batches x 4 query-quarters (3072 queries,
24 tiles of 128 per core).

Device (per core, one Bass/Tile program, SPMD):
  - One K=4 fp32 matmul per tile builds the packed selection key directly:
      nkey[p,s] = KC2 - 256*dist2(p,s) - j_s            (exact integers)
    via centered/scaled coords so every partial sum stays < 2^24 (bit-exact
    fp32; verified on HW). Larger nkey == nearer, ties broken by lower
    pixel index.
  - 4 selection rounds on the Vector engine per tile:
      m_r = reduce_max(nkey);  nkey = (nkey < m_r) * nkey
    (keys are distinct and positive, so masking the extracted max to zero
    is safe).
  - Output: the 4 winning keys per query, decoded on host into indices and
    offsets (pure elementwise output formatting).

Correctness for arbitrary inputs: after the run the host *proves* the
windowed result exact: the 4th-nearest distance returned for each query
must be strictly below the squared row-distance to the nearest candidate
row not fully covered by the window. Any failing queries (never, for
typical 5%-density inputs) are recomputed exactly.
"""

import sys

sys.path.insert(0, "/opt/trn_rl_repo")

import numpy as np

B = 2
H = 64
W = 192
N = H * W            # 12288 pixels
NUM = 4
WIN = 256            # candidates per window (j fits in 8 bits)
KC2 = 10_356_000     # key bias: > 256*max_dist2 + 255, keeps partials < 2^24
N_CORES = 8
QPC = N // 4         # queries per core (4 cores per batch)
TILES = QPC // 128   # 24 tiles of 128 queries
VALID_THRESH = 0.001

_PROGRAM = None


def _build_program():
    import concourse.bacc as bacc
    import concourse.mybir as mybir
    from concourse.tile import TileContext

    f32 = mybir.dt.float32
    nc = bacc.Bacc(None)
    lhsT_in = nc.declare_dram_parameter("lhsT_in", [4, TILES * 128], f32, isOutput=False)
    rhs_in = nc.declare_dram_parameter("rhs_in", [4, TILES * WIN], f32, isOutput=False)
    m_out = nc.declare_dram_parameter("m_out", [128, TILES * NUM], f32, isOutput=True)

    with TileContext(nc) as tc:
        with (
            tc.tile_pool(name="const", bufs=1) as const,
            tc.tile_pool(name="work", bufs=3) as work,
            tc.tile_pool(name="psum", bufs=4, space="PSUM") as psum,
        ):
            lhsT_sb = const.tile([4, TILES * 128], f32)
            rhs_sb = const.tile([4, TILES * WIN], f32)
            mr = const.tile([128, TILES * NUM], f32)
            nc.sync.dma_start(out=lhsT_sb[:, :], in_=lhsT_in[:, :])
            nc.sync.dma_start(out=rhs_sb[:, :], in_=rhs_in[:, :])

            for t in range(TILES):
                acc = psum.tile([128, WIN], f32, tag="acc")
                nc.tensor.matmul(
                    acc[:, :],
                    lhsT_sb[:, t * 128 : (t + 1) * 128],
                    rhs_sb[:, t * WIN : (t + 1) * WIN],
                    start=True,
                    stop=True,
                )
                nk0 = work.tile([128, WIN], f32, tag="nk0")
                nk1 = work.tile([128, WIN], f32, tag="nk1")
                # PSUM -> SBUF on the (otherwise idle) scalar engine
                nc.scalar.copy(nk0[:, :], acc[:, :])
                cur, bufs = nk0, [nk0, nk1]
                for r in range(NUM):
                    col = t * NUM + r
                    nc.vector.tensor_reduce(
                        out=mr[:, col : col + 1],
                        in_=cur[:, :],
                        axis=mybir.AxisListType.X,
                        op=mybir.AluOpType.max,
                    )
                    if r < NUM - 1:
                        nxt = bufs[(r + 1) % 2]
                        nc.vector.scalar_tensor_tensor(
                            out=nxt[:, :],
                            in0=cur[:, :],
                            scalar=mr[:, col : col + 1],
                            in1=cur[:, :],
                            op0=mybir.AluOpType.is_lt,
                            op1=mybir.AluOpType.mult,
                        )
                        cur = nxt

            nc.sync.dma_start(out=m_out[:, :], in_=mr[:, :])

    nc.compile()
    return nc


def _choose_window(cy, NV, y0, y1):
    """Window [lo, lo+WIN) of the y-sorted candidate list maximizing the
    minimum row-guard around query rows [y0, y1]. Returns lo."""
    if NV <= WIN:
        return 0
    los = np.arange(0, NV - WIN + 1)
    gb = np.where(los > 0, y0 - cy[np.maximum(los - 1, 0)], 10**6)
    ga = np.where(los + WIN < NV, cy[np.minimum(los + WIN, NV - 1)] - y1, 10**6)
    return int(los[np.argmin(-np.minimum(gb, ga))])


def _exact_topk_host(qn, g, cx, cy, W):
    """Exact reference-equivalent top-NUM for query pixels qn (fallback for
    queries whose window guarantee failed; not used on typical inputs)."""
    qx = qn % W
    qy = qn // W
    d = (qx[:, None] - cx[None, :]) ** 2 + (qy[:, None] - cy[None, :]) ** 2
    # lexicographic (d, g): stable argsort by g then stable by d
    order = np.argsort(d, axis=1, kind="stable")  # ties -> lower slot = lower g
    sel = order[:, :NUM]
    return g[sel], np.take_along_axis(d, sel, axis=1)


def kernel(S, xx, yy, num):
    from concourse.bass_utils import run_bass_kernel_spmd

    global _PROGRAM
    S = np.asarray(S)
    num = int(num)
    assert num == NUM and S.shape == (B, 1, H, W)

    Sf = S.reshape(B, N)
    valid = Sf > VALID_THRESH

    g_all, cx_all, cy_all = [], [], []
    for b in range(B):
        g = np.nonzero(valid[b])[0].astype(np.int64)
        if len(g) < NUM:
            raise NotImplementedError("fewer than 4 valid candidates")
        g_all.append(g)
        cx_all.append(g % W)
        cy_all.append(g // W)

    # ---- host marshaling: per-core lhsT / windowed rhs tables ----
    in_maps = []
    winlo = np.zeros((N_CORES, TILES), dtype=np.int64)
    for c in range(N_CORES):
        b, q = c // 4, c % 4
        g, cx, cy = g_all[b], cx_all[b], cy_all[b]
        NV = len(g)
        qn = np.arange(q * QPC, (q + 1) * QPC)
        qx = qn % W
        qy = qn // W
        a = 16 * qx - 1528
        ay = 16 * qy - 504
        lhsT = np.empty((4, TILES * 128), dtype=np.float32)
        lhsT[0] = -(a * a + ay * ay)
        lhsT[1] = a
        lhsT[2] = ay
        lhsT[3] = 1.0
        rhs = np.empty((4, TILES * WIN), dtype=np.float32)
        for t in range(TILES):
            lo_px = q * QPC + t * 128
            y0, y1 = lo_px // W, (lo_px + 127) // W
            lo = _choose_window(cy, NV, y0, y1)
            winlo[c, t] = lo
            src = np.minimum(lo + np.arange(WIN), NV - 1)  # clamp -> dup last
            jj = src - lo                                  # dups share j
            bb = 16 * cx[src] - 1528
            by = 16 * cy[src] - 504
            rhs[0, t * WIN : (t + 1) * WIN] = 1.0
            rhs[1, t * WIN : (t + 1) * WIN] = 2 * bb
            rhs[2, t * WIN : (t + 1) * WIN] = 2 * by
            rhs[3, t * WIN : (t + 1) * WIN] = KC2 - jj - bb * bb - by * by
        in_maps.append({"lhsT_in": lhsT, "rhs_in": rhs})

    if _PROGRAM is None:
        _PROGRAM = _build_program()
    globals()["_LAST_IN_MAPS"] = in_maps
    res = run_bass_kernel_spmd(_PROGRAM, in_maps, list(range(N_CORES)))

    # ---- decode on host (elementwise output formatting) ----
    args = np.zeros((B, NUM, N), dtype=np.int32)
    dist4 = np.zeros((B, N), dtype=np.int64)
    gsel = np.zeros((B, NUM, N), dtype=np.int64)
    for c in range(N_CORES):
        b, q = c // 4, c % 4
        g = g_all[c // 4]
        m = np.rint(res.results[c]["m_out"].astype(np.float64)).astype(np.int64)
        m = m.reshape(128, TILES, NUM)
        key = KC2 - m                     # = 256*d + j
        d = key >> 8
        j = key & 255
        s = winlo[c][None, :, None] + j   # global candidate slot
        gg = g[s]                         # [128, TILES, NUM] global pixel idx
        n_of = (q * QPC + np.arange(TILES)[None, :, None] * 128
                + np.arange(128)[:, None, None])
        for r in range(NUM):
            args[b, r, n_of[:, :, 0].ravel()] = gg[:, :, r].ravel().astype(np.int32)
            gsel[b, r, n_of[:, :, 0].ravel()] = gg[:, :, r].ravel()
        dist4[b, n_of[:, :, 0].ravel()] = d[:, :, NUM - 1].ravel()

    # ---- post-hoc exactness proof of the windowed result ----
    qn_full = np.arange(N)
    qy_full = qn_full // W
    for c in range(N_CORES):
        b, q = c // 4, c % 4
        g, cy = g_all[b], cy_all[b]
        NV = len(g)
        for t in range(TILES):
            lo = int(winlo[c, t])
            lo_px = q * QPC + t * 128
            qn = qn_full[lo_px : lo_px + 128]
            md = np.full(128, 10**12, dtype=np.int64)
            if lo > 0:
                md = np.minimum(md, (qy_full[qn] - cy[lo - 1]) ** 2)
            if lo + WIN < NV:
                md = np.minimum(md, (cy[lo + WIN] - qy_full[qn]) ** 2)
            bad = dist4[b, qn] >= md
            if bad.any():
                # exact recompute for the few failing queries
                qbad = qn[bad]
                gsel_f, d_f = _exact_topk_host(qbad, g, cx_all[b], cy_all[b], W)
                args[b, :, qbad] = gsel_f.astype(np.int32)
                gsel[b, :, qbad] = gsel_f

    # IPC offsets from the input coordinate grids (matches reference's
    # xy[:, args] - xy[:, query])
    xf = np.asarray(xx, dtype=np.float32).reshape(N)
    yf = np.asarray(yy, dtype=np.float32).reshape(N)
    IPCnum = np.empty((B, 2, NUM, N), dtype=np.float32)
    for b in range(B):
        IPCnum[b, 0] = xf[gsel[b]] - xf[None, :]
        IPCnum[b, 1] = yf[gsel[b]] - yf[None, :]

    return IPCnum, args
